# revision 33
# baseline (speedup 1.0000x reference)
"""BertScore model kernel for Trainium2 (8 NeuronCores, SPMD data-parallel).

Reference computation: cosine-normalized per-layer token reps, per-(layer,batch)
similarity matrix dots = h1 @ h2^T (256x256, contraction D=1024), ragged masked
max over rows/cols + masked means -> s1,s2, F1 harmonic mean -> (B,NL) features,
BatchNorm over batch, linear head -> (B,).

Design (v5.2), all claims HW-profiled on trn2 via NTFF traces:
- Measured structure: single-shot exec_time = ~8.7us fixed NEFF preamble +
  input-DMA span (bytes / HBM stream rate, which drifts 240-330GB/s with
  ambient device load) + ~2.6us tail. Compute engines all have slack
  (PE ~45%, DVE ~49%, ACT ~33% busy), so ONLY transferred bytes + tail
  matter. --bench For_i slope: 30987 ns/iter (prior artifact: 45132).
- fp8 e4m3 inputs (h scaled by 32; |h|<=1 so max 32 << 448 e4m3 limit) with
  DoubleRow matmuls: half the DMA bytes and PE cycles vs fp16.
- Scheduling unit = one (layer, batch) pair. The 256 units are clustered
  (simulated annealing) into 32 classes of 8 (one unit per core per class);
  a class is one program piece with compile-time (I, J) = rounded class
  maxima. A batch's 4 layer units are identical in size, so the optimizer
  co-locates big batches' units instead of letting them inflate 4 separate
  per-batch slots: 10325 columns/core vs 2728*4=10912 for per-batch slots
  (exact-length floor 8934). I rounds to 32 (DR ldweights accept %16 but PE
  out-partition bases are 32-granular and the colmaxT group-dropping needs
  %32); J is unconstrained (HW-verified odd-width DR matmul).
- DMA: consecutive classes merge into ~384-column chunks (keeps per-partition
  descriptor runs >= ~3KB; small pieces measurably drop the stream ~4%, and
  MINCOLS=640 measured worse), EXCEPT the trailing chunk, which is split one
  class per DMA: a single completion sem for several tail classes serialized
  their DVE reduce chains AFTER the stream ended (traced ~4us tail hole;
  per-class tail chunks won 4/4 ABBA rounds, -2us median). ONE queue only:
  splitting input DMAs across both HWDGE rings measured 70us vs 50us; output
  DMAs mid-stream dilute the input queue (330 -> 305-321 GB/s).
- Padding rows duplicate token row 0 (max-neutral); no device-side masks.
- Column maxes via DVE tensor_reduce(apply_transpose=True): 32x32 stream-
  transposed reduce gives per-(32-partition-group, column) partial maxes,
  combined on host which knows true lengths and drops garbage groups
  (I%32==0 keeps valid/garbage groups separate). No PE transposes at all
  (~30% of PE busy in v3.2) and a shorter per-class dependency chain.
- Outputs: bf16, packed exactly (~87KB), rowmax+colmax interleaved
  per class in ONE tensor so the tail needs exactly TWO DMAs (bulk while
  the last class computes + a tiny final piece); four separate rm/cm
  pieces cost ~0.6us serialized issue time each after the last reduce.
  Keeping the PSUM->SBUF ACT copy stage measured equal to PSUM-direct
  reduces and leaves DVE margin (ACT is off the DMA path).
"""
import os
import numpy as np

NL, B, L1, L2, D = 4, 64, 256, 256, 1024
NCORES = 8
NCLS = NL * B // NCORES   # 32 classes per core
KT = D // 128             # contraction subtiles
SCALE = 32.0              # fp8 input scale; dots come back scaled by SCALE**2
BN_EPS = 1e-8
LOGIT_SCALE = 1.0

DTYPE = os.environ.get("BSM_DTYPE", "f8")        # f8 | f16
REPEAT = int(os.environ.get("BSM_REPEAT", "1"))  # body repeats (for timing)
DENSE = int(os.environ.get("BSM_DENSE", "0"))    # 1: pad all classes to 256
SKIP = set(os.environ.get("BSM_SKIP", "").split(","))  # debug: io,mm,act,red
IOBUFS = int(os.environ.get("BSM_IOBUFS", "8"))
LOOPN = int(os.environ.get("BSM_LOOPN", "0"))  # >0: wrap body in For_i loop

_CACHE = {}


def _j32(J):
    return min(L2, (J + 31) & ~31)


def _om_offsets(classes):
    """Per-class start column in the packed combined output (2 rowmax cols
    then cI*nbj colmax-partial cols per class); returns (offs, total)."""
    offs, o = [], 0
    for (I, J) in classes:
        offs.append(o)
        cI = 2 if I > 128 else 1
        o += 2 + cI * (_j32(J) // 32)
    return offs, o


def _build(dtype_name, repeat, iobufs, classes):
    """classes: tuple of (I_k, J_k) compile-time sizes, one piece each."""
    import concourse.bacc as bacc
    import concourse.mybir as mybir
    import concourse.tile as tile

    f32 = mybir.dt.float32
    bf16 = mybir.dt.bfloat16
    dt_in = {"f8": mybir.dt.float8e4, "f16": mybir.dt.float16}[dtype_name]
    fp8 = dt_in == mybir.dt.float8e4

    nc = bacc.Bacc("TRN2", target_bir_lowering=False, debug=False,
                   num_devices=NCORES)

    # planar ragged pack, per partition p (contiguous, class-major):
    #   [class0: h1 (KT,I_0) | h2 (KT,J_0)][class1: ...] ...
    # where element (t,q,i) of class ci is h_t[l, b, i, q*128+p] * SCALE
    # for that core's (b, l) assignment
    offs = []
    W = 0
    for (I, J) in classes:
        offs.append(W)
        W += KT * (I + J)
    hbd = nc.dram_tensor("hb", [128, W], dt_in, kind="ExternalInput")
    # DMA chunking: consecutive classes merge into one dma_start until the
    # chunk reaches ~MINCOLS columns, keeping per-partition descriptor runs
    # >=2KB (small-piece DMAs measurably drop the stream rate ~4%)
    MINCOLS = 384
    chunks = []          # (start_class, n_classes)
    s0, acc = 0, 0
    for ci, (I, J) in enumerate(classes):
        acc += I + J
        if acc >= MINCOLS or ci == len(classes) - 1:
            chunks.append((s0, ci - s0 + 1))
            s0, acc = ci + 1, 0
    # the TRAILING chunk must not lump classes: a single completion sem for
    # several tail classes serializes their DVE reduce chains AFTER the
    # stream ends (traced: ~4us compute-tail hole). Split it per class so
    # each tail class computes as its own bytes land.
    if chunks and chunks[-1][1] > 1:
        c0, n = chunks.pop()
        chunks.extend((c0 + k, 1) for k in range(n))
    omoffs, NCOLO = _om_offsets(classes)
    omd = nc.dram_tensor("om", [128, NCOLO], bf16, kind="ExternalOutput")

    with tile.TileContext(nc) as tc:
        with tc.tile_pool(name="io", bufs=iobufs) as io, \
             tc.tile_pool(name="dsbp", bufs=4) as dsbp, \
             tc.tile_pool(name="accp", bufs=1) as accp, \
             tc.tile_pool(name="ps", bufs=3, space="PSUM") as ps:

            OM = accp.tile([128, NCOLO], bf16)
            if SKIP & {"io", "mm", "act", "red"}:
                nc.vector.memset(OM, 0.0)

            hbap = hbd.ap()
            vmax = mybir.AluOpType.max
            X = mybir.AxisListType.X
            IDENT = mybir.ActivationFunctionType.Identity
            DR = mybir.MatmulPerfMode.DoubleRow

            import contextlib
            loop_cm = (tc.For_i(0, LOOPN, 1,
                                hint_engines=(mybir.EngineType.PE,))
                       if LOOPN > 0 else contextlib.nullcontext())

            with loop_cm:
              for _rep in range(repeat):
                for (c0, ncl) in chunks:
                  WC = sum(KT * (I + J) for (I, J) in classes[c0:c0 + ncl])
                  hbt = io.tile([128, WC], dt_in, tag="hb")
                  if "io" not in SKIP:
                      o = offs[c0]
                      nc.sync.dma_start(out=hbt, in_=hbap[:, o:o + WC])
                  for ci in range(c0, c0 + ncl):
                    I, J = classes[ci]
                    ich = [min(128, I)] + ([I - 128] if I > 128 else [])
                    cI = len(ich)
                    J32 = _j32(J)
                    nbj = J32 // 32
                    ob = offs[ci] - offs[c0]
                    h1v = hbt[:, ob:ob + KT * I].rearrange(
                        "p (q i) -> p q i", q=KT)
                    h2v = hbt[:, ob + KT * I:ob + KT * (I + J)].rearrange(
                        "p (q j) -> p q j", q=KT)
                    if "mm" in SKIP:
                        continue
                    dps = ps.tile([128, 2, L2], f32, tag="dots")
                    for it, cw in enumerate(ich):
                        i0 = it * 128
                        if fp8:
                            for qp in range(0, KT, 2):
                                nc.tensor.matmul(
                                    out=dps[:cw, it, :J],
                                    lhsT=h1v[:, qp:qp + 2, i0:i0 + cw],
                                    rhs=h2v[:, qp:qp + 2, :],
                                    start=(qp == 0), stop=(qp == KT - 2),
                                    perf_mode=DR)
                        else:
                            for q in range(KT):
                                nc.tensor.matmul(
                                    out=dps[:cw, it, :J],
                                    lhsT=h1v[:, q, i0:i0 + cw],
                                    rhs=h2v[:, q, :],
                                    start=(q == 0), stop=(q == KT - 1))
                    if "act" in SKIP:
                        continue
                    dsb = dsbp.tile([128, 2, L2], bf16, tag="dsb")
                    nc.scalar.activation(
                        out=dsb[:, :cI, :J], in_=dps[:, :cI, :J], func=IDENT)
                    if "red" in SKIP:
                        continue
                    oc = omoffs[ci]
                    nc.vector.tensor_reduce(
                        out=OM[:, oc:oc + cI],
                        in_=dsb[:, :cI, :J], axis=X, op=vmax)
                    nc.vector.tensor_reduce(
                        out=OM[:, oc + 2:oc + 2 + cI * nbj],
                        in_=dsb[:, :cI, :J32].rearrange(
                            "p c (b w) -> p c b w", w=32),
                        axis=X, op=vmax, apply_transpose=True)

            # outputs ship at the end in exactly TWO DMAs (rowmax+colmax
            # interleaved per class): bulk (all but the last class) issues
            # while the tail computes; the final piece is tiny. Four
            # separate rm/cm pieces cost ~0.6us serialized issue time each
            # on Sync after the last reduces. (Mid-stream output DMAs would
            # dilute the saturated input queue: 330 -> 305-321 GB/s.)
            oL = omoffs[chunks[-1][0]]
            nc.sync.dma_start(out=omd.ap()[:, :oL], in_=OM[:, :oL])
            nc.sync.dma_start(out=omd.ap()[:, oL:], in_=OM[:, oL:])

    nc.finalize()
    return nc


def _rnd(x, m):
    # Stationary (I) widths: DR ldweights accept %16, but PE out-partition
    # bases are 32-granular, so partial-32 slivers of PSUM rows cannot be
    # written/neutralized without re-adding the padding bytes -> I stays %32
    # (the colmaxT host-side group dropping relies on it too). Moving (J)
    # widths are unconstrained (HW-verified J=61 DR matmul).
    return min(256, (int(x) + m - 1) & ~(m - 1))


def _assign_classes(len1, len2):
    """Cluster the NL*B (layer,batch) units into NCLS classes of NCORES
    members (one per core), minimizing sum of rounded class maxima
    (= per-core DMA bytes). Units are canonicalized to (I, J) =
    (min len, max len); a batch's 4 layer units have identical size, so
    the optimizer co-locates them. Returns (assign, classes, swapped):
    assign[ci][c] = (batch, layer) at (core c, class ci)."""
    import random
    l1 = np.asarray(len1).astype(int)
    l2 = np.asarray(len2).astype(int)
    swapped = l1 > l2
    lo = np.minimum(l1, l2)
    hi = np.maximum(l1, l2)
    units = [(int(lo[b]), int(hi[b]), b, l)
             for b in range(B) for l in range(NL)]
    if DENSE:
        assign = [[(k * NCORES + c, l) for c in range(NCORES)]
                  for l in range(NL) for k in range(B // NCORES)]
        return assign, [(L1, L2)] * NCLS, np.zeros(B, dtype=bool)

    ulo = np.array([u[0] for u in units])
    uhi = np.array([u[1] for u in units])
    order = sorted(range(len(units)), key=lambda i: -(ulo[i] + uhi[i]))
    groups = [[order[NCORES * g + c] for c in range(NCORES)]
              for g in range(NCLS)]

    def gcost(g):
        return _rnd(ulo[g].max(), 32) + _rnd(uhi[g].max(), 1)

    garr = [np.array(g) for g in groups]
    cost = [gcost(g) for g in garr]
    total = sum(cost)
    best_total = total
    best = [g.copy() for g in garr]
    rng = random.Random(0)
    ITERS = 120000
    T0 = 40.0
    for t in range(ITERS):
        T = T0 * (1 - t / ITERS) + 0.01
        ga = rng.randrange(NCLS)
        gb = rng.randrange(NCLS)
        if ga == gb:
            continue
        i = rng.randrange(NCORES)
        j = rng.randrange(NCORES)
        garr[ga][i], garr[gb][j] = garr[gb][j], garr[ga][i]
        na, nb = gcost(garr[ga]), gcost(garr[gb])
        d = na + nb - cost[ga] - cost[gb]
        if d <= 0 or rng.random() < np.exp(-d / T):
            cost[ga], cost[gb] = na, nb
            total += d
            if total < best_total:
                best_total = total
                best = [g.copy() for g in garr]
        else:
            garr[ga][i], garr[gb][j] = garr[gb][j], garr[ga][i]

    # order: DESCENDING size. Only the end of the schedule matters (the
    # DMA stream is back-to-back regardless; compute rides behind it), and
    # the post-stream tail is the last few classes' serialized DVE reduces:
    # ending with the smallest classes cuts that tail (traced: big-class
    # reduces at the end cost ~3.1us of post-stream DVE).
    costs = [gcost(g) for g in best]
    best = [best[i] for i in np.argsort(costs)[::-1]]

    classes = [(_rnd(ulo[g].max(), 32), _rnd(uhi[g].max(), 1)) for g in best]
    assign = [[(units[g[c]][2], units[g[c]][3]) for c in range(NCORES)]
              for g in best]
    return assign, classes, swapped


def _get_nc(classes):
    key = (DTYPE, REPEAT, IOBUFS, LOOPN, tuple(sorted(SKIP)), tuple(classes))
    if key not in _CACHE:
        _CACHE[key] = _build(DTYPE, REPEAT, IOBUFS, tuple(classes))
    return _CACHE[key]


def _host_prep(reps1, reps2, len1, len2, assign, classes, swapped):
    """Normalize+scale, pack the ragged planar fp8 array per core.
    Rows in [len, classmax) duplicate token row 0 (max-neutral padding).
    Batches with swapped[b] are packed h2-first (transposed orientation)."""
    import ml_dtypes
    np_in = {"f8": ml_dtypes.float8_e4m3, "f16": np.float16}[DTYPE]

    def planar(r):
        r = np.asarray(r, dtype=np.float32)
        n = np.sqrt(np.einsum('lbid,lbid->lbi', r, r))
        h = r * (SCALE / n[..., None])                # (NL, B, L, D)
        x = h.reshape(NL, B, L1, KT, 128)             # d = q*128 + p
        return x.transpose(4, 1, 0, 3, 2).astype(np_in)   # (128, B, NL, KT, L)

    p1 = planar(reps1)
    p2 = planar(reps2)
    len1 = np.asarray(len1).astype(np.int64)
    len2 = np.asarray(len2).astype(np.int64)

    W = sum(KT * (I + J) for (I, J) in classes)
    in_maps = []
    for c in range(NCORES):
        hb = np.empty((128, W), dtype=np_in)
        o = 0
        for ci, (I, J) in enumerate(classes):
            b, l = assign[ci][c]
            sides = ((p1, I, len1[b]), (p2, J, len2[b]))
            if swapped[b]:
                sides = ((p2, I, len2[b]), (p1, J, len1[b]))
            for p, n, ln in sides:
                s = p[:, b, l, :, :n].copy()          # (128, KT, n)
                s[:, :, ln:] = s[:, :, :1]            # duplicate row 0
                hb[:, o:o + KT * n] = s.reshape(128, KT * n)
                o += KT * n
        in_maps.append({"hb": hb})
    return in_maps, len1, len2


def _epilogue(results, len1, len2, w, b, assign, classes, swapped):
    """rm (128, NCLS*2) + cm (128, packed) bf16 partials per core ->
    s1,s2 -> F1 -> BatchNorm -> head."""
    maxv_rows = np.empty((NL, B, L1), dtype=np.float64)  # max over valid j
    maxv_cols = np.empty((NL, B, L2), dtype=np.float64)  # max over valid i
    omoffs, _ = _om_offsets(classes)
    for c, res in enumerate(results):
        om = np.asarray(res["om"], dtype=np.float64)  # (128, packed)
        for ci, (I, J) in enumerate(classes):
            bidx, l = assign[ci][c]
            ich = [min(128, I)] + ([I - 128] if I > 128 else [])
            nbj = _j32(J) // 32
            oc = omoffs[ci]
            rows = np.full(256, -np.inf)
            for it in range(len(ich)):
                rows[128 * it:128 * (it + 1)] = om[:, oc + it]
            # cols: partial at om[32a + j%32, oc+2 + it*nbj + j//32],
            # a in [0, ich[it]//32); combine over (it, a)
            j = np.arange(J)
            cols = np.full(256, -np.inf)
            for it, cw in enumerate(ich):
                base = oc + 2 + it * nbj + j // 32
                for a in range(cw // 32):
                    cols[:J] = np.maximum(cols[:J], om[32 * a + j % 32, base])
            if swapped[bidx]:
                maxv_cols[l, bidx] = rows
                maxv_rows[l, bidx] = cols
            else:
                maxv_rows[l, bidx] = rows
                maxv_cols[l, bidx] = cols
    inv = 1.0 / (SCALE * SCALE)
    maxv_rows *= inv
    maxv_cols *= inv

    mask1 = (np.arange(L1)[None, :] < len1[:, None])  # (B, L1)
    mask2 = (np.arange(L2)[None, :] < len2[:, None])
    n1 = len1.astype(np.float64)
    n2 = len2.astype(np.float64)

    # s2: mean over valid i of (max over valid j); s1: mean over valid j of
    # (max over valid i)
    with np.errstate(invalid="ignore"):
        s2 = np.where(mask1[None], maxv_rows, 0.0).sum(axis=2) / n1[None]
        s1 = np.where(mask2[None], maxv_cols, 0.0).sum(axis=2) / n2[None]
    feat = (2.0 * s1 * s2 / (s1 + s2)).T                    # (B, NL)
    mean = feat.mean(axis=0, keepdims=True)
    var = ((feat - mean) ** 2).mean(axis=0, keepdims=True)
    feat = (feat - mean) / np.sqrt(var + BN_EPS)
    w = np.asarray(w, dtype=np.float64)
    bb = np.asarray(b, dtype=np.float64)
    out = LOGIT_SCALE * (feat @ w.T + bb)[:, 0]
    return out.astype(np.float32)


LAST_RUN = {}


def kernel(reps1, reps2, len1, len2, w, b):
    from concourse.bass_utils import run_bass_kernel_spmd

    assign, classes, swapped = _assign_classes(len1, len2)
    nc = _get_nc(classes)
    in_maps, l1, l2 = _host_prep(reps1, reps2, len1, len2, assign, classes,
                                 swapped)
    res = run_bass_kernel_spmd(nc, in_maps, list(range(NCORES)))
    LAST_RUN["results"] = res
    LAST_RUN["in_maps"] = in_maps
    LAST_RUN["nc"] = nc
    LAST_RUN["slots"] = classes
    return _epilogue(res.results, l1, l2, w, b, assign, classes, swapped)


# revision 35
# speedup vs baseline: 1.0192x; 1.0192x over previous
"""BertScore model kernel for Trainium2 (8 NeuronCores, SPMD data-parallel).

Reference computation: cosine-normalized per-layer token reps, per-(layer,batch)
similarity matrix dots = h1 @ h2^T (256x256, contraction D=1024), ragged masked
max over rows/cols + masked means -> s1,s2, F1 harmonic mean -> (B,NL) features,
BatchNorm over batch, linear head -> (B,).

Design (v5.2), all claims HW-profiled on trn2 via NTFF traces:
- Measured structure: single-shot exec_time = ~8.7us fixed NEFF preamble +
  input-DMA span (bytes / HBM stream rate, which drifts 240-330GB/s with
  ambient device load) + ~2.6us tail. Compute engines all have slack
  (PE ~45%, DVE ~49%, ACT ~33% busy), so ONLY transferred bytes + tail
  matter. --bench For_i slope: 30987 ns/iter (prior artifact: 45132).
- fp8 e4m3 inputs (h scaled by 32; |h|<=1 so max 32 << 448 e4m3 limit) with
  DoubleRow matmuls: half the DMA bytes and PE cycles vs fp16.
- Scheduling unit = one (layer, batch) pair. The 256 units are clustered
  (simulated annealing) into 32 classes of 8 (one unit per core per class);
  a class is one program piece with compile-time (I, J) = rounded class
  maxima. A batch's 4 layer units are identical in size, so the optimizer
  co-locates big batches' units instead of letting them inflate 4 separate
  per-batch slots: 10325 columns/core vs 2728*4=10912 for per-batch slots
  (exact-length floor 8934). I rounds to 32 (DR ldweights accept %16 but PE
  out-partition bases are 32-granular and the colmaxT group-dropping needs
  %32); J is unconstrained (HW-verified odd-width DR matmul).
- DMA: consecutive classes merge into ~384-column chunks (keeps per-partition
  descriptor runs >= ~3KB; small pieces measurably drop the stream ~4%, and
  MINCOLS=640 measured worse), EXCEPT the trailing chunk, which is split one
  class per DMA: a single completion sem for several tail classes serialized
  their DVE reduce chains AFTER the stream ended (traced ~4us tail hole;
  per-class tail chunks won 4/4 ABBA rounds, -2us median). ONE queue only:
  splitting input DMAs across both HWDGE rings measured 70us vs 50us; output
  DMAs mid-stream dilute the input queue (330 -> 305-321 GB/s).
- Padding rows duplicate token row 0 (max-neutral); no device-side masks.
- Column maxes via DVE tensor_reduce(apply_transpose=True): 32x32 stream-
  transposed reduce gives per-(32-partition-group, column) partial maxes,
  combined on host which knows true lengths and drops garbage groups
  (I%32==0 keeps valid/garbage groups separate). No PE transposes at all
  (~30% of PE busy in v3.2) and a shorter per-class dependency chain.
- Outputs: bf16, packed exactly (~87KB), rowmax+colmax interleaved
  per class in ONE tensor so the tail needs exactly TWO DMAs (bulk while
  the last class computes + a tiny final piece); four separate rm/cm
  pieces cost ~0.6us serialized issue time each after the last reduce.
  Bulk classes keep the PSUM->SBUF ACT copy stage (leaves DVE margin; ACT
  is off the DMA path); per-class TAIL classes reduce from PSUM directly -
  the tail is a DVE pileup (tiny classes arrive faster than their chains
  drain) and dropping the ACT hop there measured med 44.7 vs 45.6us.
"""
import os
import numpy as np

NL, B, L1, L2, D = 4, 64, 256, 256, 1024
NCORES = 8
NCLS = NL * B // NCORES   # 32 classes per core
KT = D // 128             # contraction subtiles
SCALE = 32.0              # fp8 input scale; dots come back scaled by SCALE**2
BN_EPS = 1e-8
LOGIT_SCALE = 1.0

DTYPE = os.environ.get("BSM_DTYPE", "f8")        # f8 | f16
REPEAT = int(os.environ.get("BSM_REPEAT", "1"))  # body repeats (for timing)
DENSE = int(os.environ.get("BSM_DENSE", "0"))    # 1: pad all classes to 256
SKIP = set(os.environ.get("BSM_SKIP", "").split(","))  # debug: io,mm,act,red
IOBUFS = int(os.environ.get("BSM_IOBUFS", "8"))
LOOPN = int(os.environ.get("BSM_LOOPN", "0"))  # >0: wrap body in For_i loop

_CACHE = {}


def _j32(J):
    return min(L2, (J + 31) & ~31)


def _om_offsets(classes):
    """Per-class start column in the packed combined output (2 rowmax cols
    then cI*nbj colmax-partial cols per class); returns (offs, total)."""
    offs, o = [], 0
    for (I, J) in classes:
        offs.append(o)
        cI = 2 if I > 128 else 1
        o += 2 + cI * (_j32(J) // 32)
    return offs, o


def _build(dtype_name, repeat, iobufs, classes):
    """classes: tuple of (I_k, J_k) compile-time sizes, one piece each."""
    import concourse.bacc as bacc
    import concourse.mybir as mybir
    import concourse.tile as tile

    f32 = mybir.dt.float32
    bf16 = mybir.dt.bfloat16
    dt_in = {"f8": mybir.dt.float8e4, "f16": mybir.dt.float16}[dtype_name]
    fp8 = dt_in == mybir.dt.float8e4

    nc = bacc.Bacc("TRN2", target_bir_lowering=False, debug=False,
                   num_devices=NCORES)

    # planar ragged pack, per partition p (contiguous, class-major):
    #   [class0: h1 (KT,I_0) | h2 (KT,J_0)][class1: ...] ...
    # where element (t,q,i) of class ci is h_t[l, b, i, q*128+p] * SCALE
    # for that core's (b, l) assignment
    offs = []
    W = 0
    for (I, J) in classes:
        offs.append(W)
        W += KT * (I + J)
    hbd = nc.dram_tensor("hb", [128, W], dt_in, kind="ExternalInput")
    # DMA chunking: consecutive classes merge into one dma_start until the
    # chunk reaches ~MINCOLS columns, keeping per-partition descriptor runs
    # >=2KB (small-piece DMAs measurably drop the stream rate ~4%)
    MINCOLS = 384
    chunks = []          # (start_class, n_classes)
    s0, acc = 0, 0
    for ci, (I, J) in enumerate(classes):
        acc += I + J
        if acc >= MINCOLS or ci == len(classes) - 1:
            chunks.append((s0, ci - s0 + 1))
            s0, acc = ci + 1, 0
    # the TRAILING chunk must not lump classes: a single completion sem for
    # several tail classes serializes their DVE reduce chains AFTER the
    # stream ends (traced: ~4us compute-tail hole). Split it per class so
    # each tail class computes as its own bytes land.
    if chunks and chunks[-1][1] > 1:
        c0, n = chunks.pop()
        chunks.extend((c0 + k, 1) for k in range(n))
    tail0 = chunks[-1][0] if chunks and chunks[-1][1] == 1 else NCLS
    for (tc0, tn) in reversed(chunks):
        if tn == 1:
            tail0 = tc0
        else:
            break
    omoffs, NCOLO = _om_offsets(classes)
    omd = nc.dram_tensor("om", [128, NCOLO], bf16, kind="ExternalOutput")

    with tile.TileContext(nc) as tc:
        with tc.tile_pool(name="io", bufs=iobufs) as io, \
             tc.tile_pool(name="dsbp", bufs=4) as dsbp, \
             tc.tile_pool(name="accp", bufs=1) as accp, \
             tc.tile_pool(name="ps", bufs=3, space="PSUM") as ps:

            OM = accp.tile([128, NCOLO], bf16)
            if SKIP & {"io", "mm", "act", "red"}:
                nc.vector.memset(OM, 0.0)

            hbap = hbd.ap()
            vmax = mybir.AluOpType.max
            X = mybir.AxisListType.X
            IDENT = mybir.ActivationFunctionType.Identity
            DR = mybir.MatmulPerfMode.DoubleRow

            import contextlib
            loop_cm = (tc.For_i(0, LOOPN, 1,
                                hint_engines=(mybir.EngineType.PE,))
                       if LOOPN > 0 else contextlib.nullcontext())

            with loop_cm:
              for _rep in range(repeat):
                for (c0, ncl) in chunks:
                  WC = sum(KT * (I + J) for (I, J) in classes[c0:c0 + ncl])
                  hbt = io.tile([128, WC], dt_in, tag="hb")
                  if "io" not in SKIP:
                      o = offs[c0]
                      nc.sync.dma_start(out=hbt, in_=hbap[:, o:o + WC])
                  for ci in range(c0, c0 + ncl):
                    I, J = classes[ci]
                    ich = [min(128, I)] + ([I - 128] if I > 128 else [])
                    cI = len(ich)
                    J32 = _j32(J)
                    nbj = J32 // 32
                    ob = offs[ci] - offs[c0]
                    h1v = hbt[:, ob:ob + KT * I].rearrange(
                        "p (q i) -> p q i", q=KT)
                    h2v = hbt[:, ob + KT * I:ob + KT * (I + J)].rearrange(
                        "p (q j) -> p q j", q=KT)
                    if "mm" in SKIP:
                        continue
                    dps = ps.tile([128, 2, L2], f32, tag="dots")
                    for it, cw in enumerate(ich):
                        i0 = it * 128
                        if fp8:
                            for qp in range(0, KT, 2):
                                nc.tensor.matmul(
                                    out=dps[:cw, it, :J],
                                    lhsT=h1v[:, qp:qp + 2, i0:i0 + cw],
                                    rhs=h2v[:, qp:qp + 2, :],
                                    start=(qp == 0), stop=(qp == KT - 2),
                                    perf_mode=DR)
                        else:
                            for q in range(KT):
                                nc.tensor.matmul(
                                    out=dps[:cw, it, :J],
                                    lhsT=h1v[:, q, i0:i0 + cw],
                                    rhs=h2v[:, q, :],
                                    start=(q == 0), stop=(q == KT - 1))
                    if "act" in SKIP:
                        continue
                    if ci < tail0:
                        dsb = dsbp.tile([128, 2, L2], bf16, tag="dsb")
                        nc.scalar.activation(
                            out=dsb[:, :cI, :J], in_=dps[:, :cI, :J],
                            func=IDENT)
                    else:
                        # per-class tail region: reduce from PSUM directly
                        # (HW-verified) - each tail chain loses the ACT hop,
                        # draining the end-of-stream DVE pileup sooner
                        dsb = dps
                    if "red" in SKIP:
                        continue
                    oc = omoffs[ci]
                    nc.vector.tensor_reduce(
                        out=OM[:, oc:oc + cI],
                        in_=dsb[:, :cI, :J], axis=X, op=vmax)
                    nc.vector.tensor_reduce(
                        out=OM[:, oc + 2:oc + 2 + cI * nbj],
                        in_=dsb[:, :cI, :J32].rearrange(
                            "p c (b w) -> p c b w", w=32),
                        axis=X, op=vmax, apply_transpose=True)

            # outputs ship at the end in exactly TWO DMAs (rowmax+colmax
            # interleaved per class): bulk (all but the last class) issues
            # while the tail computes; the final piece is tiny. Four
            # separate rm/cm pieces cost ~0.6us serialized issue time each
            # on Sync after the last reduces. (Mid-stream output DMAs would
            # dilute the saturated input queue: 330 -> 305-321 GB/s.)
            oL = omoffs[chunks[-1][0]]
            nc.sync.dma_start(out=omd.ap()[:, :oL], in_=OM[:, :oL])
            nc.sync.dma_start(out=omd.ap()[:, oL:], in_=OM[:, oL:])

    nc.finalize()
    return nc


def _rnd(x, m):
    # Stationary (I) widths: DR ldweights accept %16, but PE out-partition
    # bases are 32-granular, so partial-32 slivers of PSUM rows cannot be
    # written/neutralized without re-adding the padding bytes -> I stays %32
    # (the colmaxT host-side group dropping relies on it too). Moving (J)
    # widths are unconstrained (HW-verified J=61 DR matmul).
    return min(256, (int(x) + m - 1) & ~(m - 1))


def _assign_classes(len1, len2):
    """Cluster the NL*B (layer,batch) units into NCLS classes of NCORES
    members (one per core), minimizing sum of rounded class maxima
    (= per-core DMA bytes). Units are canonicalized to (I, J) =
    (min len, max len); a batch's 4 layer units have identical size, so
    the optimizer co-locates them. Returns (assign, classes, swapped):
    assign[ci][c] = (batch, layer) at (core c, class ci)."""
    import random
    l1 = np.asarray(len1).astype(int)
    l2 = np.asarray(len2).astype(int)
    swapped = l1 > l2
    lo = np.minimum(l1, l2)
    hi = np.maximum(l1, l2)
    units = [(int(lo[b]), int(hi[b]), b, l)
             for b in range(B) for l in range(NL)]
    if DENSE:
        assign = [[(k * NCORES + c, l) for c in range(NCORES)]
                  for l in range(NL) for k in range(B // NCORES)]
        return assign, [(L1, L2)] * NCLS, np.zeros(B, dtype=bool)

    ulo = np.array([u[0] for u in units])
    uhi = np.array([u[1] for u in units])
    order = sorted(range(len(units)), key=lambda i: -(ulo[i] + uhi[i]))
    groups = [[order[NCORES * g + c] for c in range(NCORES)]
              for g in range(NCLS)]

    def gcost(g):
        return _rnd(ulo[g].max(), 32) + _rnd(uhi[g].max(), 1)

    garr = [np.array(g) for g in groups]
    cost = [gcost(g) for g in garr]
    total = sum(cost)
    best_total = total
    best = [g.copy() for g in garr]
    rng = random.Random(0)
    ITERS = 120000
    T0 = 40.0
    for t in range(ITERS):
        T = T0 * (1 - t / ITERS) + 0.01
        ga = rng.randrange(NCLS)
        gb = rng.randrange(NCLS)
        if ga == gb:
            continue
        i = rng.randrange(NCORES)
        j = rng.randrange(NCORES)
        garr[ga][i], garr[gb][j] = garr[gb][j], garr[ga][i]
        na, nb = gcost(garr[ga]), gcost(garr[gb])
        d = na + nb - cost[ga] - cost[gb]
        if d <= 0 or rng.random() < np.exp(-d / T):
            cost[ga], cost[gb] = na, nb
            total += d
            if total < best_total:
                best_total = total
                best = [g.copy() for g in garr]
        else:
            garr[ga][i], garr[gb][j] = garr[gb][j], garr[ga][i]

    # order: DESCENDING size. Only the end of the schedule matters (the
    # DMA stream is back-to-back regardless; compute rides behind it), and
    # the post-stream tail is the last few classes' serialized DVE reduces:
    # ending with the smallest classes cuts that tail (traced: big-class
    # reduces at the end cost ~3.1us of post-stream DVE).
    costs = [gcost(g) for g in best]
    best = [best[i] for i in np.argsort(costs)[::-1]]

    classes = [(_rnd(ulo[g].max(), 32), _rnd(uhi[g].max(), 1)) for g in best]
    assign = [[(units[g[c]][2], units[g[c]][3]) for c in range(NCORES)]
              for g in best]
    return assign, classes, swapped


def _get_nc(classes):
    key = (DTYPE, REPEAT, IOBUFS, LOOPN, tuple(sorted(SKIP)), tuple(classes))
    if key not in _CACHE:
        _CACHE[key] = _build(DTYPE, REPEAT, IOBUFS, tuple(classes))
    return _CACHE[key]


def _host_prep(reps1, reps2, len1, len2, assign, classes, swapped):
    """Normalize+scale, pack the ragged planar fp8 array per core.
    Rows in [len, classmax) duplicate token row 0 (max-neutral padding).
    Batches with swapped[b] are packed h2-first (transposed orientation)."""
    import ml_dtypes
    np_in = {"f8": ml_dtypes.float8_e4m3, "f16": np.float16}[DTYPE]

    def planar(r):
        r = np.asarray(r, dtype=np.float32)
        n = np.sqrt(np.einsum('lbid,lbid->lbi', r, r))
        h = r * (SCALE / n[..., None])                # (NL, B, L, D)
        x = h.reshape(NL, B, L1, KT, 128)             # d = q*128 + p
        return x.transpose(4, 1, 0, 3, 2).astype(np_in)   # (128, B, NL, KT, L)

    p1 = planar(reps1)
    p2 = planar(reps2)
    len1 = np.asarray(len1).astype(np.int64)
    len2 = np.asarray(len2).astype(np.int64)

    W = sum(KT * (I + J) for (I, J) in classes)
    in_maps = []
    for c in range(NCORES):
        hb = np.empty((128, W), dtype=np_in)
        o = 0
        for ci, (I, J) in enumerate(classes):
            b, l = assign[ci][c]
            sides = ((p1, I, len1[b]), (p2, J, len2[b]))
            if swapped[b]:
                sides = ((p2, I, len2[b]), (p1, J, len1[b]))
            for p, n, ln in sides:
                s = p[:, b, l, :, :n].copy()          # (128, KT, n)
                s[:, :, ln:] = s[:, :, :1]            # duplicate row 0
                hb[:, o:o + KT * n] = s.reshape(128, KT * n)
                o += KT * n
        in_maps.append({"hb": hb})
    return in_maps, len1, len2


def _epilogue(results, len1, len2, w, b, assign, classes, swapped):
    """rm (128, NCLS*2) + cm (128, packed) bf16 partials per core ->
    s1,s2 -> F1 -> BatchNorm -> head."""
    maxv_rows = np.empty((NL, B, L1), dtype=np.float64)  # max over valid j
    maxv_cols = np.empty((NL, B, L2), dtype=np.float64)  # max over valid i
    omoffs, _ = _om_offsets(classes)
    for c, res in enumerate(results):
        om = np.asarray(res["om"], dtype=np.float64)  # (128, packed)
        for ci, (I, J) in enumerate(classes):
            bidx, l = assign[ci][c]
            ich = [min(128, I)] + ([I - 128] if I > 128 else [])
            nbj = _j32(J) // 32
            oc = omoffs[ci]
            rows = np.full(256, -np.inf)
            for it in range(len(ich)):
                rows[128 * it:128 * (it + 1)] = om[:, oc + it]
            # cols: partial at om[32a + j%32, oc+2 + it*nbj + j//32],
            # a in [0, ich[it]//32); combine over (it, a)
            j = np.arange(J)
            cols = np.full(256, -np.inf)
            for it, cw in enumerate(ich):
                base = oc + 2 + it * nbj + j // 32
                for a in range(cw // 32):
                    cols[:J] = np.maximum(cols[:J], om[32 * a + j % 32, base])
            if swapped[bidx]:
                maxv_cols[l, bidx] = rows
                maxv_rows[l, bidx] = cols
            else:
                maxv_rows[l, bidx] = rows
                maxv_cols[l, bidx] = cols
    inv = 1.0 / (SCALE * SCALE)
    maxv_rows *= inv
    maxv_cols *= inv

    mask1 = (np.arange(L1)[None, :] < len1[:, None])  # (B, L1)
    mask2 = (np.arange(L2)[None, :] < len2[:, None])
    n1 = len1.astype(np.float64)
    n2 = len2.astype(np.float64)

    # s2: mean over valid i of (max over valid j); s1: mean over valid j of
    # (max over valid i)
    with np.errstate(invalid="ignore"):
        s2 = np.where(mask1[None], maxv_rows, 0.0).sum(axis=2) / n1[None]
        s1 = np.where(mask2[None], maxv_cols, 0.0).sum(axis=2) / n2[None]
    feat = (2.0 * s1 * s2 / (s1 + s2)).T                    # (B, NL)
    mean = feat.mean(axis=0, keepdims=True)
    var = ((feat - mean) ** 2).mean(axis=0, keepdims=True)
    feat = (feat - mean) / np.sqrt(var + BN_EPS)
    w = np.asarray(w, dtype=np.float64)
    bb = np.asarray(b, dtype=np.float64)
    out = LOGIT_SCALE * (feat @ w.T + bb)[:, 0]
    return out.astype(np.float32)


LAST_RUN = {}


def kernel(reps1, reps2, len1, len2, w, b):
    from concourse.bass_utils import run_bass_kernel_spmd

    assign, classes, swapped = _assign_classes(len1, len2)
    nc = _get_nc(classes)
    in_maps, l1, l2 = _host_prep(reps1, reps2, len1, len2, assign, classes,
                                 swapped)
    res = run_bass_kernel_spmd(nc, in_maps, list(range(NCORES)))
    LAST_RUN["results"] = res
    LAST_RUN["in_maps"] = in_maps
    LAST_RUN["nc"] = nc
    LAST_RUN["slots"] = classes
    return _epilogue(res.results, l1, l2, w, b, assign, classes, swapped)
